# revision 14
# baseline (speedup 1.0000x reference)
"""EventImage Fusion Block — Trainium2 Bass kernel.

Data-parallel over batch: 8 batches -> 8 NeuronCores, identical SPMD program.
Per core all activations live in "T layout" [channel(partition), pixel(free)].

Key tricks:
  - All three LayerNorms: stats via ones-matmul on PE (mean arrives already
    broadcast across partitions); rstd = exp(-0.5*ln(var+eps)) on ACT so only
    the natural_log_exp table set is needed (one switch to gelu at the end).
  - l2norm over pixels is never materialized: accumulate S = q k^T,
    diag(q q^T), diag(k k^T) in PSUM over the whole pixel range, then rescale
    S by the rstd's before softmax.
  - Attention apply + output projection fused: W2 = Wo @ blockdiag(attn) is
    computed on device (2 tiny matmuls), so the per-pixel attention work is a
    single dense [256,256] matmul against v.
"""

import sys

sys.path.insert(0, "/opt/trn_rl_repo")

import numpy as np

B, C, H, W = 8, 256, 128, 128
HEADS = 8
HIDDEN = 2 * C
N = H * W          # 16384 pixels per batch
P = 128            # partitions
KB = C // P        # 2 channel blocks
NT = 512           # pixel tile
NTILES = N // NT   # 32
NSUB = NT // P     # 4 subtiles of 128 px
MB_H = HIDDEN // P # 4 hidden blocks
LN_EPS = 1e-5

_CACHE = {}
LAST_EXEC_NS = None
LAST_NC = None
LAST_IN_MAPS = None


def _build(use_eq, use_ek, use_ev):
    import concourse.bass as bass
    import concourse.tile as tile
    from concourse import bacc, mybir

    F32 = mybir.dt.float32
    BF16 = mybir.dt.bfloat16
    AF = mybir.ActivationFunctionType
    OP = mybir.AluOpType


    nc = bacc.Bacc("TRN2", target_bir_lowering=False)
    GPE = nc.vector  # elementwise offload engine (gpsimd deadlocks the tile sim)

    def din(name, shape):
        return nc.declare_dram_parameter(name, list(shape), F32, isOutput=False)

    img_d = din("img", (C, N))
    evt_d = din("evt", (C, N))
    wqT_d = din("wqT", (C, C))
    wkT_d = din("wkT", (C, C))
    wvT_d = din("wvT", (C, C))
    woT_d = din("woT", (C, C))
    f1T_d = din("f1T", (C, HIDDEN))
    f2T_d = din("f2T", (HIDDEN, C))
    b1c_d = din("b1c", (P, MB_H))
    b2c_d = din("b2c", (P, KB))
    tau_d = din("tau", (P, KB))
    onesf_d = din("onesf", (P, P))       # all 1/C
    onesr_d = din("onesr", (1, P))       # all 1.0
    id_d = din("id128", (P, P))
    mask_d = din("maskbd", (P, P))
    nbig_d = din("negbig", (P, P))
    eq_d = din("eqb", (P, C)) if use_eq else None      # e_q row-broadcast
    ek_d = din("ekb", (P, C)) if use_ek else None
    ev_d = din("evv", (P, KB)) if use_ev else None
    out_d = nc.declare_dram_parameter("out", [C, N], F32, isOutput=True)

    with tile.TileContext(nc) as tc:
        cst = tc.alloc_tile_pool(name="cst", bufs=1)
        big = tc.alloc_tile_pool(name="big", bufs=1)

        def load_const(dram, shape, tag):
            t = cst.tile(list(shape), F32, tag=tag)
            nc.gpsimd.dma_start(out=t[:], in_=dram[:])
            return t

        def load_w(dram, nblk, f, tag):
            t = cst.tile([P, nblk, f], F32, tag=tag)
            for a in range(nblk):
                nc.gpsimd.dma_start(out=t[:, a], in_=dram[a * P : (a + 1) * P, :])
            return t

        wqT = load_w(wqT_d, KB, C, "wqT")   # [128, c_blk, 256]
        wkT = load_w(wkT_d, KB, C, "wkT")
        wvT = load_w(wvT_d, KB, C, "wvT")
        woT = load_w(woT_d, KB, C, "woT")
        f1T = load_w(f1T_d, KB, HIDDEN, "f1T")
        f2T = load_w(f2T_d, MB_H, C, "f2T")
        b1c = load_const(b1c_d, (P, MB_H), "b1c")
        b2c = load_const(b2c_d, (P, KB), "b2c")
        tau = load_const(tau_d, (P, KB), "tauc")
        onesf = load_const(onesf_d, (P, P), "onesf")
        onesr = load_const(onesr_d, (1, P), "onesr")
        id128 = load_const(id_d, (P, P), "id128")
        maskbd = load_const(mask_d, (P, P), "maskbd")
        negbig = load_const(nbig_d, (P, P), "negbig")
        eqc = load_const(eq_d, (P, C), "eqc") if use_eq else None
        ekc = load_const(ek_d, (P, C), "ekc") if use_ek else None
        evc = load_const(ev_d, (P, KB), "evc") if use_ev else None

        # DRAM views as [blk][...]
        def dview(d):  # [C, N] -> [KB][128, N]
            return [d[kb * P : (kb + 1) * P, :] for kb in range(KB)]

        img_v = dview(img_d)
        evt_v = dview(evt_d)
        out_v = dview(out_d)

        # v persists across phases (bf16): [128, KB, N] = 64KB/partition
        v_sb = big.tile([P, KB, N], BF16)
        eps_col = big.tile([P, 1], F32)
        nc.vector.memset(eps_col[:], LN_EPS)
        # LN2 stats rows, per-tile columns packed: [32, NT] on partitions 0..31
        rstd_rows = big.tile([NTILES, NT], F32)
        srow_rows = big.tile([NTILES, NT], F32)

        gram_ps = tc.alloc_tile_pool(name="gram_ps", bufs=1, space="PSUM")
        SQ_ps = gram_ps.tile([P, 2 * C], F32)   # cols 0:256 = S, 256:512 = QQ
        KK_ps = gram_ps.tile([P, C], F32)

        # -------------------- Phase 1: q/k grams + v --------------------
        with tc.tile_pool(name="l1", bufs=3) as l1, \
             tc.tile_pool(name="l1s", bufs=3) as l1s, \
             tc.tile_pool(name="qk", bufs=3) as qkp, \
             tc.tile_pool(name="st_ps", bufs=3, space="PSUM") as st_ps, \
             tc.tile_pool(name="qk_ps", bufs=2, space="PSUM") as qk_ps, \
             tc.tile_pool(name="v_ps", bufs=1, space="PSUM") as v_ps:

            for i in range(NTILES):
                px = slice(i * NT, (i + 1) * NT)
                first = i == 0
                last = i == NTILES - 1

                def norm_input(src_v, tag):
                    raw = l1.tile([P, KB, NT], F32, tag=f"raw{tag}")
                    for kb in range(KB):
                        nc.gpsimd.dma_start(out=raw[:, kb], in_=src_v[kb][:, px])
                    sq = l1.tile([P, KB, NT], F32, tag="sq")
                    for kb in range(KB):
                        GPE.tensor_mul(sq[:, kb], raw[:, kb], raw[:, kb])
                    mu_ps = st_ps.tile([P, NT], F32, tag="st")
                    msq_ps = st_ps.tile([P, NT], F32, tag="st")
                    for kb in range(KB):
                        nc.tensor.matmul(mu_ps[:], onesf[:], raw[:, kb],
                                         start=(kb == 0), stop=(kb == KB - 1))
                    for kb in range(KB):
                        nc.tensor.matmul(msq_ps[:], onesf[:], sq[:, kb],
                                         start=(kb == 0), stop=(kb == KB - 1))
                    musq = l1s.tile([P, NT], F32, tag="musq")
                    nc.scalar.activation(musq[:], mu_ps[:], AF.Square)
                    var = l1s.tile([P, NT], F32, tag="var")
                    nc.vector.tensor_tensor(var[:], msq_ps[:], musq[:], OP.subtract)
                    lnv = l1s.tile([P, NT], F32, tag="lnv")
                    nc.scalar.activation(lnv[:], var[:], AF.Ln, bias=eps_col[:])
                    rstd = l1s.tile([P, NT], F32, tag="rstd")
                    nc.scalar.activation(rstd[:], lnv[:], AF.Exp, scale=-0.5)
                    xn = l1.tile([P, KB, NT], F32, tag=f"xn{tag}")
                    for kb in range(KB):
                        nc.vector.tensor_tensor(xn[:, kb], raw[:, kb], mu_ps[:],
                                                OP.subtract)
                        GPE.tensor_mul(xn[:, kb], xn[:, kb], rstd[:])
                    return raw, xn

                _, x_t = norm_input(img_v, "x")
                _, y_t = norm_input(evt_v, "y")

                # v^T tile (bf16 into persistent buffer)
                for mb in range(KB):
                    vp = v_ps.tile([P, NT], F32, tag="v")
                    for kb in range(KB):
                        nc.tensor.matmul(vp[:], wvT[:, kb, mb * P : (mb + 1) * P],
                                         y_t[:, kb], start=(kb == 0),
                                         stop=(kb == KB - 1))
                    if use_ev:
                        nc.vector.tensor_scalar(
                            v_sb[:, mb, px], vp[:], evc[:, mb : mb + 1], None,
                            OP.add)
                    else:
                        nc.scalar.copy(v_sb[:, mb, px], vp[:])

                # q, k pixel-major + gram accumulation
                for s in range(NSUB):
                    sub = slice(s * P, (s + 1) * P)
                    qp = qk_ps.tile([P, C], F32, tag="qk")
                    kp = qk_ps.tile([P, C], F32, tag="qk")
                    for kb in range(KB):
                        nc.tensor.matmul(qp[:], x_t[:, kb, sub], wqT[:, kb],
                                         start=(kb == 0), stop=(kb == KB - 1))
                    for kb in range(KB):
                        nc.tensor.matmul(kp[:], y_t[:, kb, sub], wkT[:, kb],
                                         start=(kb == 0), stop=(kb == KB - 1))
                    q_sb = qkp.tile([P, C], F32, tag="qs")
                    k_sb = qkp.tile([P, C], F32, tag="ks")
                    nc.scalar.copy(q_sb[:], qp[:])
                    nc.scalar.copy(k_sb[:], kp[:])
                    if use_eq:
                        nc.vector.tensor_tensor(q_sb[:], q_sb[:], eqc[:], OP.add)
                    if use_ek:
                        nc.vector.tensor_tensor(k_sb[:], k_sb[:], ekc[:], OP.add)
                    gfirst = first and s == 0
                    glast = last and s == NSUB - 1
                    for mb in range(KB):
                        blk = slice(mb * P, (mb + 1) * P)
                        nc.tensor.matmul(SQ_ps[:, blk], q_sb[:, blk], k_sb[:, blk],
                                         start=gfirst, stop=glast,
                                         skip_group_check=True)
                        nc.tensor.matmul(SQ_ps[:, C + mb * P : C + (mb + 1) * P],
                                         q_sb[:, blk], q_sb[:, blk],
                                         start=gfirst, stop=glast,
                                         skip_group_check=True)
                        nc.tensor.matmul(KK_ps[:, blk], k_sb[:, blk], k_sb[:, blk],
                                         start=gfirst, stop=glast,
                                         skip_group_check=True)

        # -------------------- Phase 2: softmax + W2 --------------------
        with tc.tile_pool(name="mid", bufs=1) as mid, \
             tc.tile_pool(name="mid_ps", bufs=1, space="PSUM") as mid_ps:
            s_sb = mid.tile([P, 2 * C], F32)
            kk_sb = mid.tile([P, C], F32)
            nc.vector.tensor_copy(s_sb[:], SQ_ps[:])
            nc.vector.tensor_copy(kk_sb[:], KK_ps[:])

            # diag(QQ), diag(KK) -> [128, 2] each
            dcols = mid.tile([P, 4], F32)  # dq0 dq1 dk0 dk1
            tmpd = mid.tile([P, P], F32)
            for b in range(KB):
                nc.vector.tensor_tensor(
                    tmpd[:], s_sb[:, C + b * P : C + (b + 1) * P], id128[:], OP.mult)
                nc.vector.reduce_sum(dcols[:, b : b + 1], tmpd[:],
                                     axis=mybir.AxisListType.X)
            for b in range(KB):
                nc.vector.tensor_tensor(
                    tmpd[:], kk_sb[:, b * P : (b + 1) * P], id128[:], OP.mult)
                nc.vector.reduce_sum(dcols[:, 2 + b : 3 + b], tmpd[:],
                                     axis=mybir.AxisListType.X)
            lncols = mid.tile([P, 4], F32)
            nc.scalar.activation(lncols[:], dcols[:], AF.Ln, bias=0.0)
            rcols = mid.tile([P, 4], F32)
            nc.scalar.activation(rcols[:], lncols[:], AF.Exp, scale=-0.5)
            # rq * tau
            rq2 = mid.tile([P, KB], F32)
            nc.vector.tensor_tensor(rq2[:], rcols[:, 0:KB], tau[:], OP.mult)

            # A = S * rq2(col, per block)
            A = mid.tile([P, C], F32)
            for b in range(KB):
                nc.vector.tensor_scalar(
                    A[:, b * P : (b + 1) * P], s_sb[:, b * P : (b + 1) * P],
                    rq2[:, b : b + 1], None, OP.mult)
            # rstd_k broadcast along columns: per block transpose row + bcast
            rkb_ps = mid_ps.tile([P, C], F32)
            for b in range(KB):
                rkT_ps = mid_ps.tile([1, P], F32, tag="rkT")
                nc.tensor.transpose(rkT_ps[:], rcols[:, KB + b : KB + b + 1],
                                    id128[:])
                rk_row = mid.tile([1, P], F32, tag="rkrow")
                nc.vector.tensor_copy(rk_row[:], rkT_ps[:])
                nc.tensor.matmul(rkb_ps[:, b * P : (b + 1) * P], onesr[:],
                                 rk_row[:], start=True, stop=True)
            nc.vector.tensor_tensor(A[:], A[:], rkb_ps[:], OP.mult)
            # mask to block-diagonal
            A3 = A.rearrange("p (b q) -> p b q", b=KB)
            nc.vector.tensor_tensor(
                A3, A3, maskbd[:, None, :].to_broadcast((P, KB, P)), OP.mult)
            nc.vector.tensor_tensor(
                A3, A3, negbig[:, None, :].to_broadcast((P, KB, P)), OP.add)
            # softmax over last dim
            mx = mid.tile([P, KB], F32)
            nc.vector.reduce_max(mx[:], A3, axis=mybir.AxisListType.X)
            nmx = mid.tile([P, KB], F32)
            nc.vector.tensor_scalar(nmx[:], mx[:], -1.0, None, OP.mult)
            E = mid.tile([P, C], F32)
            for b in range(KB):
                nc.scalar.activation(E[:, b * P : (b + 1) * P],
                                     A[:, b * P : (b + 1) * P], AF.Exp,
                                     bias=nmx[:, b : b + 1])
            sm = mid.tile([P, KB], F32)
            nc.vector.reduce_sum(sm[:], E.rearrange("p (b q) -> p b q", b=KB),
                                 axis=mybir.AxisListType.X)
            rs = mid.tile([P, KB], F32)
            nc.vector.reciprocal(rs[:], sm[:])
            attn = mid.tile([P, C], F32)
            for b in range(KB):
                nc.vector.tensor_scalar(
                    attn[:, b * P : (b + 1) * P], E[:, b * P : (b + 1) * P],
                    rs[:, b : b + 1], None, OP.mult)
            # W2T[vb] = attn_blk(vb)^T-contract woT rows of block vb  (bf16)
            w2_sb = big.tile([P, KB, C], BF16)
            for vb in range(KB):
                w2p = mid_ps.tile([P, C], F32, tag="w2p")
                nc.tensor.matmul(w2p[:], attn[:, vb * P : (vb + 1) * P],
                                 woT[:, vb], start=True, stop=True)
                nc.scalar.copy(w2_sb[:, vb], w2p[:])

        gram_ps.release()

        # -------------------- helper for fused tile --------------------
        def make_fused(i, pool, ps_pool, tag):
            px = slice(i * NT, (i + 1) * NT)
            img2 = pool.tile([P, KB, NT], F32, tag=f"img{tag}")
            for kb in range(KB):
                nc.gpsimd.dma_start(out=img2[:, kb], in_=img_v[kb][:, px])
            fused = pool.tile([P, KB, NT], F32, tag=f"fus{tag}")
            for mb in range(KB):
                o3 = ps_pool.tile([P, NT], F32, tag="o3")
                for kb in range(KB):
                    nc.tensor.matmul(o3[:], w2_sb[:, kb, mb * P : (mb + 1) * P],
                                     v_sb[:, kb, px], start=(kb == 0),
                                     stop=(kb == KB - 1))
                nc.vector.tensor_tensor(fused[:, mb], img2[:, mb], o3[:], OP.add)
            return fused

        # -------------------- Phase 3a: LN2 stats --------------------
        with tc.tile_pool(name="l2a", bufs=3) as l2a, \
             tc.tile_pool(name="l2as", bufs=3) as l2as, \
             tc.tile_pool(name="o3_ps", bufs=2, space="PSUM") as o3_ps, \
             tc.tile_pool(name="st2_ps", bufs=2, space="PSUM") as st2_ps:
            onesc = big.tile([P, 1], F32)
            nc.vector.memset(onesc[:], 1.0 / C)
            for i in range(NTILES):
                fused = make_fused(i, l2a, o3_ps, "a")
                fsq = l2a.tile([P, KB, NT], F32, tag="fsq")
                for kb in range(KB):
                    GPE.tensor_mul(fsq[:, kb], fused[:, kb], fused[:, kb])
                mu_ps = st2_ps.tile([1, NT], F32, tag="mu2")
                msq_ps = st2_ps.tile([1, NT], F32, tag="msq2")
                for kb in range(KB):
                    nc.tensor.matmul(mu_ps[:], onesc[:], fused[:, kb],
                                     start=(kb == 0), stop=(kb == KB - 1))
                for kb in range(KB):
                    nc.tensor.matmul(msq_ps[:], onesc[:], fsq[:, kb],
                                     start=(kb == 0), stop=(kb == KB - 1))
                musq = l2as.tile([1, NT], F32, tag="musq2")
                nc.scalar.activation(musq[:], mu_ps[:], AF.Square)
                var = l2as.tile([1, NT], F32, tag="var2")
                nc.vector.tensor_tensor(var[:], msq_ps[:], musq[:], OP.subtract)
                lnv = l2as.tile([1, NT], F32, tag="lnv2")
                nc.scalar.activation(lnv[:], var[:], AF.Ln, bias=eps_col[0:1])
                rstd = l2as.tile([1, NT], F32, tag="rstd2")
                nc.scalar.activation(rstd[:], lnv[:], AF.Exp, scale=-0.5)
                srow = l2as.tile([1, NT], F32, tag="srow2")
                nc.vector.tensor_tensor(srow[:], mu_ps[:], rstd[:], OP.mult)
                nc.gpsimd.dma_start(out=rstd_rows[i : i + 1, :], in_=rstd[:])
                nc.gpsimd.dma_start(out=srow_rows[i : i + 1, :], in_=srow[:])

        # -------------------- Phase 3b: FFN --------------------
        with tc.tile_pool(name="l2b", bufs=3) as l2b, \
             tc.tile_pool(name="o3b_ps", bufs=2, space="PSUM") as o3b_ps, \
             tc.tile_pool(name="bc_ps", bufs=1, space="PSUM") as bc_ps, \
             tc.tile_pool(name="mm_ps", bufs=3, space="PSUM") as mm_ps:
            for i in range(NTILES):
                px = slice(i * NT, (i + 1) * NT)
                fused = make_fused(i, l2b, o3b_ps, "b")
                rb_ps = bc_ps.tile([P, NT], F32, tag="rb")
                sb_ps = bc_ps.tile([P, NT], F32, tag="sb")
                stage_r = l2b.tile([1, NT], F32, tag="str")
                stage_s = l2b.tile([1, NT], F32, tag="sts")
                nc.gpsimd.dma_start(out=stage_r[:], in_=rstd_rows[i : i + 1, :])
                nc.gpsimd.dma_start(out=stage_s[:], in_=srow_rows[i : i + 1, :])
                nc.tensor.matmul(rb_ps[:], onesr[:], stage_r[:],
                                 start=True, stop=True)
                nc.tensor.matmul(sb_ps[:], onesr[:], stage_s[:],
                                 start=True, stop=True)
                z = l2b.tile([P, KB, NT], F32, tag="z")
                for kb in range(KB):
                    nc.vector.tensor_tensor(z[:, kb], fused[:, kb], rb_ps[:],
                                            OP.mult)
                    nc.vector.tensor_tensor(z[:, kb], z[:, kb], sb_ps[:],
                                            OP.subtract)
                z1 = l2b.tile([P, MB_H, NT], F32, tag="z1")
                for mb in range(MB_H):
                    zp = mm_ps.tile([P, NT], F32, tag="mm")
                    for kb in range(KB):
                        nc.tensor.matmul(zp[:], f1T[:, kb, mb * P : (mb + 1) * P],
                                         z[:, kb], start=(kb == 0),
                                         stop=(kb == KB - 1))
                    nc.scalar.activation(z1[:, mb], zp[:], AF.Gelu,
                                         bias=b1c[:, mb : mb + 1])
                outt = l2b.tile([P, KB, NT], F32, tag="outt")
                for mb in range(KB):
                    zp = mm_ps.tile([P, NT], F32, tag="mm")
                    for kb in range(MB_H):
                        nc.tensor.matmul(zp[:], f2T[:, kb, mb * P : (mb + 1) * P],
                                         z1[:, kb], start=(kb == 0),
                                         stop=(kb == MB_H - 1))
                    z2 = l2b.tile([P, NT], F32, tag="z2")
                    nc.scalar.activation(z2[:], zp[:], AF.Gelu,
                                         bias=b2c[:, mb : mb + 1])
                    GPE.tensor_add(outt[:, mb], fused[:, mb], z2[:])
                for mb in range(KB):
                    nc.gpsimd.dma_start(out=out_v[mb][:, px], in_=outt[:, mb])

        big.release()
        cst.release()

    nc.compile()
    return nc


def _get_nc(flags):
    if flags not in _CACHE:
        _CACHE[flags] = _build(*flags)
    return _CACHE[flags]


def kernel(**inputs):
    from concourse.bass_utils import run_bass_kernel_spmd

    f32 = lambda x: np.ascontiguousarray(np.asarray(x, dtype=np.float32))
    img = f32(inputs["image"]).reshape(B, C, N)
    evt = f32(inputs["event"]).reshape(B, C, N)
    w_i, b_i = f32(inputs["ln_img_w"]), f32(inputs["ln_img_b"])
    w_e, b_e = f32(inputs["ln_evt_w"]), f32(inputs["ln_evt_b"])
    Wq, Wk, Wv, Wo = (f32(inputs[k]) for k in ("Wq", "Wk", "Wv", "Wo"))
    tau = f32(inputs["temperature"]).reshape(HEADS)
    w2n, b2n = f32(inputs["norm2_w"]), f32(inputs["norm2_b"])
    fc1_w, fc1_b = f32(inputs["fc1_w"]), f32(inputs["fc1_b"])
    fc2_w, fc2_b = f32(inputs["fc2_w"]), f32(inputs["fc2_b"])

    wqT = np.ascontiguousarray(Wq.T * w_i[:, None])
    wkT = np.ascontiguousarray(Wk.T * w_e[:, None])
    wvT = np.ascontiguousarray(Wv.T * w_e[:, None])
    woT = np.ascontiguousarray(Wo.T)
    f1T = np.ascontiguousarray((fc1_w * w2n[None, :]).T)   # [C, HIDDEN]
    b1p = fc1_w @ b2n + fc1_b
    f2T = np.ascontiguousarray(fc2_w.T)                    # [HIDDEN, C]
    b1c = np.ascontiguousarray(b1p.reshape(MB_H, P).T)
    b2c = np.ascontiguousarray(fc2_b.reshape(KB, P).T)
    tauc = np.ascontiguousarray(np.repeat(tau, C // HEADS).reshape(KB, P).T)
    e_q = Wq @ b_i
    e_k = Wk @ b_e
    e_v = Wv @ b_e
    use_eq = bool(np.any(e_q != 0))
    use_ek = bool(np.any(e_k != 0))
    use_ev = bool(np.any(e_v != 0))

    idx = np.arange(P)
    mask = (idx[:, None] // (C // HEADS) == idx[None, :] // (C // HEADS))
    consts = {
        "wqT": wqT, "wkT": wkT, "wvT": wvT, "woT": woT,
        "f1T": f1T, "f2T": f2T, "b1c": b1c, "b2c": b2c, "tau": tauc,
        "onesf": np.full((P, P), 1.0 / C, np.float32),
        "onesr": np.ones((1, P), np.float32),
        "id128": np.eye(P, dtype=np.float32),
        "maskbd": mask.astype(np.float32),
        "negbig": ((1.0 - mask) * -1e30).astype(np.float32),
    }
    if use_eq:
        consts["eqb"] = np.broadcast_to(e_q, (P, C)).copy()
    if use_ek:
        consts["ekb"] = np.broadcast_to(e_k, (P, C)).copy()
    if use_ev:
        consts["evv"] = np.ascontiguousarray(e_v.reshape(KB, P).T)
    consts = {k: np.ascontiguousarray(v, dtype=np.float32)
              for k, v in consts.items()}

    nc = _get_nc((use_eq, use_ek, use_ev))
    in_maps = [dict(consts, img=img[b], evt=evt[b]) for b in range(B)]
    global LAST_NC, LAST_IN_MAPS
    LAST_NC, LAST_IN_MAPS = nc, in_maps
    import os
    trace = bool(os.environ.get("BASS_KERNEL_TRACE"))
    res = run_bass_kernel_spmd(nc, in_maps, list(range(B)), trace=trace)
    global LAST_EXEC_NS
    if getattr(res, "exec_time_ns", None):
        LAST_EXEC_NS = res.exec_time_ns
    out = np.stack([res.results[b]["out"] for b in range(B)])
    return out.reshape(B, C, H, W).astype(np.float32)
